# revision 25
# baseline (speedup 1.0000x reference)
"""ChebyKAN linear layer on 8 Trainium2 NeuronCores.

Math: y[b,o] = sum_{i,d} T_d(w[b,i]) * C[i,o,d], with w = tanh(tanh(x)) and
T_d the Chebyshev polynomials (cos(d*arccos(w)) == T_d(w) for |w|<=1).

The ACT engine has no arccos/cos, so the device evaluates the Chebyshev-product
basis phi = [T1, T1^2, T1*T2, T2^2, T2*T3, T3^2, T3*T4, T4^2] built from ACT
Square + DVE multiplies (T2, T4 and the T3 helper come from cheap
tensor_scalar affine ops). Via T_{2k} = 2*T_k^2-1 and T_{m+n} = 2*T_m*T_n -
T_{m-n}, an exact host-side linear transform maps Chebyshev coefficients onto
this basis with O(1) conditioning; the constant column folds into a per-o bias
added during PSUM evacuation. (A plain monomial basis w^j would be ~15x less
accurate here: its coefficient transform amplifies the f32r input rounding.)

Sharding: data-parallel over batch b (16384 -> 2048/core); coeffs replicated.
x is pre-transposed on the host so the contraction dim (c_in) lands on SBUF
partitions; the kernel computes y^T per core and the host transposes back.

Matmuls run in float32r (1 cycle/row at N=512, vs 4 for plain fp32; the f32r
operand grid is an 11-bit mantissa). Basis intermediates stay full fp32 and
each matmul operand is rounded to f32r exactly once - compounding f32r
roundings through the squaring chain costs ~15x in accuracy.

All input tiles are prefetched up-front on the HWDGE sync ring in PE
consumption order (every dma_start costs ~0.6us of issue time on its
sequencer, and the Tile scheduler will head-of-line block an in-order engine
queue on a late transfer if any compute op in the steady state depends on a
DMA). Output stores and non-critical loads ride the otherwise-idle SWDGE
(gpsimd) ring. A short burst of dummy matmuls on a memset tile warms the PE's
HAM clock gate (1.2 -> 2.4GHz) before the first real matmul is ready; in each
phase the last row-block runs oc-major so the four PSUM accumulation groups
finish staggered and evacuation overlaps the matmul stream.
"""

import sys

if "/opt/trn_rl_repo" not in sys.path:
    sys.path.append("/opt/trn_rl_repo")

import numpy as np

import concourse.bacc as bacc
import concourse.tile as tile
from concourse import mybir
from concourse.bass_utils import run_bass_kernel_spmd

DEGREE = 8
B, C_IN, C_OUT = 16384, 512, 512
N_CORES = 8
NB = B // N_CORES            # 2048 batch rows per core
B_TILE = 512                 # batch window per PSUM accumulation phase
N_PHASES = NB // B_TILE      # 4
N_IB = C_IN // 128           # 4 contraction row-blocks
N_J = DEGREE                 # basis funcs phi_1..phi_8 (constant -> bias)
F32 = mybir.dt.float32
F32R = mybir.dt.float32r

_CACHE = {}


def _build():
    nc = bacc.Bacc("TRN2", target_bir_lowering=False, debug=False)
    xt = nc.dram_tensor("xt", [C_IN, NB], F32, kind="ExternalInput")
    wmat = nc.dram_tensor("wmat", [C_IN, N_J * C_OUT], F32R, kind="ExternalInput")
    biasv = nc.dram_tensor("biasv", [128, 4], F32, kind="ExternalInput")
    yt = nc.dram_tensor("yt", [C_OUT, NB], F32, kind="ExternalOutput")

    Tanh = mybir.ActivationFunctionType.Tanh
    Square = mybir.ActivationFunctionType.Square
    Identity = mybir.ActivationFunctionType.Identity
    ALU_MULT = mybir.AluOpType.mult
    ALU_ADD = mybir.AluOpType.add

    with tile.TileContext(nc) as tc:
        with (
            tc.tile_pool(name="const", bufs=1) as const_pool,
            tc.tile_pool(name="wts", bufs=1) as wpool,
            tc.tile_pool(name="pows", bufs=2) as ppool,
            tc.tile_pool(name="outs", bufs=2) as opool,
            tc.tile_pool(name="psum", bufs=2, space="PSUM") as pspool,
        ):
            # PE warm-up fodder: the HAM clock gate keeps the PE at 1.2GHz
            # until ~3.4us of sustained activity; dummy matmuls on a memset
            # tile warm it up before the first real matmul is ready (~15us),
            # so the real stream runs at 2.4GHz from the start.
            dummy = const_pool.tile([128, B_TILE], F32, tag="dummy")
            nc.vector.memset(dummy[:], 0.0)
            dps = pspool.tile([128, B_TILE], F32, tag="ps3", name="dps")
            for _ in range(12):
                nc.tensor.matmul(
                    dps[:, 0:128], lhsT=dummy[:, 0:128], rhs=dummy[:, 0:128],
                    start=True, stop=True,
                )

            # Prefetch EVERYTHING on the sync (HWDGE) ring in consumption
            # order (~0.6us issue cost each). After this, no op in the phase
            # loop depends on a DMA, so the scheduler's in-order engine
            # queues cannot head-of-line block on a late transfer.
            xlbs = []
            xlb0 = ppool.tile([128, N_IB, B_TILE], F32, tag="xlb0", bufs=1)
            # first row-block sliver alone: unblocks the tanh chain ~3us
            # earlier than waiting for the full 1MB phase-0 load
            nc.sync.dma_start(
                out=xlb0[:, 0, :],
                in_=xt.ap()[0:128, 0:B_TILE],
            )
            xlbs.append(xlb0)

            w_sb = {}

            def load_w(ib, j):
                wc = wpool.tile(
                    [128, C_OUT], F32R, tag=f"wc{ib}_{j}", name=f"wc{ib}_{j}"
                )
                nc.sync.dma_start(
                    out=wc[:],
                    in_=wmat.ap()[
                        ib * 128 : (ib + 1) * 128, j * C_OUT : (j + 1) * C_OUT
                    ],
                )
                w_sb[ib, j] = wc

            # interleave by need-time: phase-0 HBM bandwidth is the scarce
            # resource, so later phases' activation loads ride after the W
            # rows the PE consumes first
            def load_xlb(ph):
                xlb = ppool.tile(
                    [128, N_IB, B_TILE], F32, tag=f"xlb{ph}", bufs=1,
                    name=f"xlb{ph}",
                )
                bsl = slice(ph * B_TILE, (ph + 1) * B_TILE)
                nc.sync.dma_start(
                    out=xlb[:],
                    in_=xt.ap()[:, bsl].rearrange("(ib p) b -> p ib b", p=128),
                )
                xlbs.append(xlb)

            load_w(0, 0)
            load_w(0, 1)
            load_w(0, 2)
            # remaining phase-0 row-blocks as slivers spread between W
            # chunks: each is needed one tanh-chain later, and a single
            # 768KB load here would stall the W trickle the PE is eating
            for ib in range(1, N_IB):
                nc.sync.dma_start(
                    out=xlb0[:, ib, :],
                    in_=xt.ap()[ib * 128 : (ib + 1) * 128, 0:B_TILE],
                )
                load_w(0, 2 * ib + 1)
                if 2 * ib + 2 < N_J:
                    load_w(0, 2 * ib + 2)
            for ib in range(N_IB):
                for j in range(N_J):
                    if (ib, j) not in w_sb:
                        load_w(ib, j)
                if ib == 2:
                    load_xlb(1)
            load_xlb(2)
            load_xlb(3)

            def w_chunk(ib, j, oc):
                return w_sb[ib, j][:, oc * 128 : (oc + 1) * 128]

            bias_t = const_pool.tile([128, 4], F32)
            nc.gpsimd.dma_start(out=bias_t[:], in_=biasv.ap())

            for ph in range(N_PHASES):
                ps = [
                    pspool.tile([128, B_TILE], F32, tag=f"ps{oc}", name=f"ps{oc}_{ph}")
                    for oc in range(4)
                ]
                bsl = slice(ph * B_TILE, (ph + 1) * B_TILE)
                xlb = xlbs[ph]
                for ib in range(N_IB):
                    # Chebyshev-product basis, full-fp32 chain
                    nc.scalar.activation(xlb[:, ib, :], xlb[:, ib, :], Tanh)
                    t1 = ppool.tile([128, B_TILE], F32, tag="t1")
                    nc.scalar.activation(t1[:], xlb[:, ib, :], Tanh)
                    f2 = ppool.tile([128, B_TILE], F32, tag="f2")
                    t2 = ppool.tile([128, B_TILE], F32, tag="t2")
                    u3 = ppool.tile([128, B_TILE], F32, tag="u3")
                    t3 = ppool.tile([128, B_TILE], F32, tag="t3")
                    f4 = ppool.tile([128, B_TILE], F32, tag="f4")
                    t4 = ppool.tile([128, B_TILE], F32, tag="t4")
                    nc.scalar.activation(f2[:], t1[:], Square)
                    nc.vector.tensor_scalar(t2[:], f2[:], 2.0, -1.0, ALU_MULT, ALU_ADD)
                    nc.vector.tensor_scalar(u3[:], f2[:], 4.0, -3.0, ALU_MULT, ALU_ADD)
                    nc.vector.tensor_mul(t3[:], t1[:], u3[:])
                    nc.scalar.activation(f4[:], t2[:], Square)
                    nc.vector.tensor_scalar(t4[:], f4[:], 2.0, -1.0, ALU_MULT, ALU_ADD)
                    # f32r-rounded matmul operands (one rounding each)
                    t1r = ppool.tile([128, B_TILE], F32R, tag="t1r", bufs=3)
                    f2r = ppool.tile([128, B_TILE], F32R, tag="f2r", bufs=3)
                    f3 = ppool.tile([128, B_TILE], F32R, tag="f3", bufs=3)
                    f4r = ppool.tile([128, B_TILE], F32R, tag="f4r", bufs=3)
                    f5 = ppool.tile([128, B_TILE], F32R, tag="f5", bufs=3)
                    f6 = ppool.tile([128, B_TILE], F32R, tag="f6", bufs=3)
                    f7 = ppool.tile([128, B_TILE], F32R, tag="f7", bufs=3)
                    f8 = ppool.tile([128, B_TILE], F32R, tag="f8", bufs=3)
                    nc.gpsimd.tensor_copy(t1r[:], t1[:])
                    nc.gpsimd.tensor_copy(f2r[:], f2[:])
                    nc.vector.tensor_mul(f3[:], t1[:], t2[:])
                    nc.gpsimd.tensor_copy(f4r[:], f4[:])
                    nc.vector.tensor_mul(f5[:], t2[:], t3[:])
                    nc.scalar.activation(f6[:], t3[:], Square)
                    nc.vector.tensor_mul(f7[:], t3[:], t4[:])
                    nc.scalar.activation(f8[:], t4[:], Square)
                    chunks = [t1r, f2r, f3, f4r, f5, f6, f7, f8]
                    if ib < N_IB - 1:
                        order = [(j, oc) for j in range(N_J) for oc in range(4)]
                    else:
                        # oc-major on the last row-block: accumulation groups
                        # finish staggered -> evacuation overlaps matmuls
                        order = [(j, oc) for oc in range(4) for j in range(N_J)]
                    for j, oc in order:
                        nc.tensor.matmul(
                            ps[oc][:],
                            lhsT=w_chunk(ib, j, oc),
                            rhs=chunks[j][:],
                            start=(ib == 0 and j == 0),
                            stop=(ib == N_IB - 1 and j == N_J - 1),
                        )
                        if ib == N_IB - 1 and j == N_J - 1:
                            osb = opool.tile(
                                [128, B_TILE], F32, tag=f"osb{oc}", name=f"osb{oc}"
                            )
                            nc.scalar.activation(
                                osb[:], ps[oc][:], Identity,
                                bias=bias_t[:, oc : oc + 1],
                            )
                            out_eng = (
                                nc.sync if ph == N_PHASES - 1 else nc.gpsimd
                            )
                            out_eng.dma_start(
                                out=yt.ap()[oc * 128 : (oc + 1) * 128, bsl],
                                in_=osb[:],
                            )
    nc.compile()
    return nc


def _host_transform(cheby_coeffs):
    # Map Chebyshev coefficients onto the device phi basis:
    # phi = [T1, T1^2, T1*T2, T2^2, T2*T3, T3^2, T3*T4, T4^2] and a constant.
    # T_{2k} = 2*T_k^2 - 1, T_{m+n} = 2*T_m*T_n - T_{m-n} =>
    #   y = bias + (C1-C3-C5-C7)*T1 + sum_{d=2..8} 2*C_d * phi_{d-1}
    #   bias_o = sum_i (C0 - C2 - C4 - C6 - C8)
    C64 = cheby_coeffs.astype(np.float64)
    bias = (C64[..., 0] - C64[..., 2] - C64[..., 4] - C64[..., 6] - C64[..., 8]).sum(
        axis=0
    )
    W = np.empty((C_IN, C_OUT, N_J), np.float64)
    W[..., 0] = C64[..., 1] - C64[..., 3] - C64[..., 5] - C64[..., 7]
    for d in range(2, DEGREE + 1):
        W[..., d - 1] = 2.0 * C64[..., d]
    # [i, j*512+o]: per-partition-contiguous coefficient rows
    Wd = np.ascontiguousarray(
        W.transpose(0, 2, 1).reshape(C_IN, N_J * C_OUT).astype(np.float32)
    )
    bias_dev = np.ascontiguousarray(bias.reshape(4, 128).T.astype(np.float32))
    return Wd, bias_dev


def kernel(x, cheby_coeffs):
    x = np.asarray(x, dtype=np.float32)
    cheby_coeffs = np.asarray(cheby_coeffs, dtype=np.float32)
    if "nc" not in _CACHE:
        _CACHE["nc"] = _build()
    nc = _CACHE["nc"]

    Wd, bias_dev = _host_transform(cheby_coeffs)
    xT = np.ascontiguousarray(x.T)                       # [c_in, b]
    in_maps = [
        {
            "xt": np.ascontiguousarray(xT[:, c * NB : (c + 1) * NB]),
            "wmat": Wd,
            "biasv": bias_dev,
        }
        for c in range(N_CORES)
    ]
    res = run_bass_kernel_spmd(nc, in_maps, core_ids=list(range(N_CORES)))
    y = np.concatenate([res.results[c]["yt"].T for c in range(N_CORES)], axis=0)
    return y


# revision 26
# speedup vs baseline: 1.2668x; 1.2668x over previous
"""ChebyKAN linear layer on 8 Trainium2 NeuronCores.

Math: y[b,o] = sum_{i,d} T_d(w[b,i]) * C[i,o,d], with w = tanh(tanh(x)) and
T_d the Chebyshev polynomials (cos(d*arccos(w)) == T_d(w) for |w|<=1).

The ACT engine has no arccos/cos, so the device evaluates the Chebyshev-product
basis phi = [T1, T1^2, T1*T2, T2^2, T2*T3, T3^2, T3*T4, T4^2] built from ACT
Square + DVE multiplies (T2, T4 and the T3 helper come from cheap
tensor_scalar affine ops). Via T_{2k} = 2*T_k^2-1 and T_{m+n} = 2*T_m*T_n -
T_{m-n}, an exact host-side linear transform maps Chebyshev coefficients onto
this basis with O(1) conditioning; the constant column folds into a per-o bias
added during PSUM evacuation. (A plain monomial basis w^j would be ~15x less
accurate here: its coefficient transform amplifies the f32r input rounding.)

Sharding: data-parallel over batch b (16384 -> 2048/core); coeffs replicated.
x is pre-transposed on the host so the contraction dim (c_in) lands on SBUF
partitions; the kernel computes y^T per core and the host transposes back.

Matmuls run in float32r (1 cycle/row at N=512, vs 4 for plain fp32; the f32r
operand grid is an 11-bit mantissa). Basis intermediates stay full fp32 and
each matmul operand is rounded to f32r exactly once - compounding f32r
roundings through the squaring chain costs ~15x in accuracy.

All input tiles are prefetched up-front on the HWDGE sync ring in PE
consumption order (every dma_start costs ~0.6us of issue time on its
sequencer, and the Tile scheduler will head-of-line block an in-order engine
queue on a late transfer if any compute op in the steady state depends on a
DMA). Output stores and non-critical loads ride the otherwise-idle SWDGE
(gpsimd) ring. A short burst of dummy matmuls on a memset tile warms the PE's
HAM clock gate (1.2 -> 2.4GHz) before the first real matmul is ready; in each
phase the last row-block runs oc-major so the four PSUM accumulation groups
finish staggered and evacuation overlaps the matmul stream.
"""

import sys

if "/opt/trn_rl_repo" not in sys.path:
    sys.path.append("/opt/trn_rl_repo")

import numpy as np

import concourse.bacc as bacc
import concourse.tile as tile
from concourse import mybir
from concourse.bass_utils import run_bass_kernel_spmd

DEGREE = 8
B, C_IN, C_OUT = 16384, 512, 512
N_CORES = 8
NB = B // N_CORES            # 2048 batch rows per core
B_TILE = 512                 # batch window per PSUM accumulation phase
N_PHASES = NB // B_TILE      # 4
N_IB = C_IN // 128           # 4 contraction row-blocks
N_J = DEGREE                 # basis funcs phi_1..phi_8 (constant -> bias)
F32 = mybir.dt.float32
F32R = mybir.dt.float32r

_CACHE = {}


def _build():
    nc = bacc.Bacc("TRN2", target_bir_lowering=False, debug=False)
    xt = nc.dram_tensor("xt", [C_IN, NB], F32, kind="ExternalInput")
    wmat = nc.dram_tensor("wmat", [C_IN, N_J * C_OUT], F32R, kind="ExternalInput")
    biasv = nc.dram_tensor("biasv", [128, 4], F32, kind="ExternalInput")
    yt = nc.dram_tensor("yt", [C_OUT, NB], F32, kind="ExternalOutput")

    Tanh = mybir.ActivationFunctionType.Tanh
    Square = mybir.ActivationFunctionType.Square
    Identity = mybir.ActivationFunctionType.Identity
    ALU_MULT = mybir.AluOpType.mult
    ALU_ADD = mybir.AluOpType.add

    with tile.TileContext(nc) as tc:
        with (
            tc.tile_pool(name="const", bufs=1) as const_pool,
            tc.tile_pool(name="wts", bufs=1) as wpool,
            tc.tile_pool(name="pows", bufs=2) as ppool,
            tc.tile_pool(name="outs", bufs=2) as opool,
            tc.tile_pool(name="psum", bufs=2, space="PSUM") as pspool,
        ):
            # PE warm-up fodder: the HAM clock gate keeps the PE at 1.2GHz
            # until ~3.4us of sustained activity; dummy matmuls on a memset
            # tile warm it up before the first real matmul is ready (~15us),
            # so the real stream runs at 2.4GHz from the start.
            dummy = const_pool.tile([128, B_TILE], F32, tag="dummy")
            nc.vector.memset(dummy[:], 0.0)
            dps = pspool.tile([128, B_TILE], F32, tag="ps3", name="dps")
            for _ in range(12):
                nc.tensor.matmul(
                    dps[:, 0:128], lhsT=dummy[:, 0:128], rhs=dummy[:, 0:128],
                    start=True, stop=True,
                )

            # Prefetch EVERYTHING on the sync (HWDGE) ring in consumption
            # order (~0.6us issue cost each). After this, no op in the phase
            # loop depends on a DMA, so the scheduler's in-order engine
            # queues cannot head-of-line block on a late transfer.
            xlbs = []
            xlb0 = ppool.tile([128, N_IB, B_TILE], F32, tag="xlb0", bufs=1)
            # first row-block sliver alone: unblocks the tanh chain ~3us
            # earlier than waiting for the full 1MB phase-0 load
            nc.sync.dma_start(
                out=xlb0[:, 0, :],
                in_=xt.ap()[0:128, 0:B_TILE],
            )
            xlbs.append(xlb0)

            w_sb = {}

            def load_w(ib, j):
                wc = wpool.tile(
                    [128, C_OUT], F32R, tag=f"wc{ib}_{j}", name=f"wc{ib}_{j}"
                )
                nc.sync.dma_start(
                    out=wc[:],
                    in_=wmat.ap()[
                        ib * 128 : (ib + 1) * 128, j * C_OUT : (j + 1) * C_OUT
                    ],
                )
                w_sb[ib, j] = wc

            # interleave by need-time: phase-0 HBM bandwidth is the scarce
            # resource, so later phases' activation loads ride after the W
            # rows the PE consumes first
            def load_xlb(ph):
                xlb = ppool.tile(
                    [128, N_IB, B_TILE], F32, tag=f"xlb{ph}", bufs=1,
                    name=f"xlb{ph}",
                )
                bsl = slice(ph * B_TILE, (ph + 1) * B_TILE)
                nc.sync.dma_start(
                    out=xlb[:],
                    in_=xt.ap()[:, bsl].rearrange("(ib p) b -> p ib b", p=128),
                )
                xlbs.append(xlb)

            load_w(0, 0)
            load_w(0, 1)
            load_w(0, 2)
            # remaining phase-0 row-blocks as slivers spread between W
            # chunks: each is needed one tanh-chain later, and a single
            # 768KB load here would stall the W trickle the PE is eating
            for ib in range(1, N_IB):
                nc.sync.dma_start(
                    out=xlb0[:, ib, :],
                    in_=xt.ap()[ib * 128 : (ib + 1) * 128, 0:B_TILE],
                )
                load_w(0, 2 * ib + 1)
                if 2 * ib + 2 < N_J:
                    load_w(0, 2 * ib + 2)
            for ib in range(N_IB):
                for j in range(N_J):
                    if (ib, j) not in w_sb:
                        load_w(ib, j)
                if ib == 2:
                    load_xlb(1)
            load_xlb(2)
            load_xlb(3)

            def w_chunk(ib, j, oc):
                return w_sb[ib, j][:, oc * 128 : (oc + 1) * 128]

            bias_t = const_pool.tile([128, 4], F32)
            nc.gpsimd.dma_start(out=bias_t[:], in_=biasv.ap())

            for ph in range(N_PHASES):
                ps = [
                    pspool.tile([128, B_TILE], F32, tag=f"ps{oc}", name=f"ps{oc}_{ph}")
                    for oc in range(4)
                ]
                bsl = slice(ph * B_TILE, (ph + 1) * B_TILE)
                xlb = xlbs[ph]
                for ib in range(N_IB):
                    # Chebyshev-product basis, full-fp32 chain
                    nc.scalar.activation(xlb[:, ib, :], xlb[:, ib, :], Tanh)
                    t1 = ppool.tile([128, B_TILE], F32, tag="t1")
                    nc.scalar.activation(t1[:], xlb[:, ib, :], Tanh)
                    f2 = ppool.tile([128, B_TILE], F32, tag="f2")
                    t2 = ppool.tile([128, B_TILE], F32, tag="t2")
                    u3 = ppool.tile([128, B_TILE], F32, tag="u3")
                    t3 = ppool.tile([128, B_TILE], F32, tag="t3")
                    f4 = ppool.tile([128, B_TILE], F32, tag="f4")
                    t4 = ppool.tile([128, B_TILE], F32, tag="t4")
                    nc.scalar.activation(f2[:], t1[:], Square)
                    nc.vector.tensor_scalar(t2[:], f2[:], 2.0, -1.0, ALU_MULT, ALU_ADD)
                    nc.vector.tensor_scalar(u3[:], f2[:], 4.0, -3.0, ALU_MULT, ALU_ADD)
                    nc.vector.tensor_mul(t3[:], t1[:], u3[:])
                    nc.scalar.activation(f4[:], t2[:], Square)
                    nc.vector.tensor_scalar(t4[:], f4[:], 2.0, -1.0, ALU_MULT, ALU_ADD)
                    # f32r-rounded matmul operands (one rounding each)
                    t1r = ppool.tile([128, B_TILE], F32R, tag="t1r", bufs=3)
                    f2r = ppool.tile([128, B_TILE], F32R, tag="f2r", bufs=3)
                    f3 = ppool.tile([128, B_TILE], F32R, tag="f3", bufs=3)
                    f4r = ppool.tile([128, B_TILE], F32R, tag="f4r", bufs=3)
                    f5 = ppool.tile([128, B_TILE], F32R, tag="f5", bufs=3)
                    f6 = ppool.tile([128, B_TILE], F32R, tag="f6", bufs=3)
                    f7 = ppool.tile([128, B_TILE], F32R, tag="f7", bufs=3)
                    f8 = ppool.tile([128, B_TILE], F32R, tag="f8", bufs=3)
                    nc.vector.tensor_copy(t1r[:], t1[:])
                    nc.vector.tensor_copy(f2r[:], f2[:])
                    nc.vector.tensor_mul(f3[:], t1[:], t2[:])
                    nc.vector.tensor_copy(f4r[:], f4[:])
                    nc.vector.tensor_mul(f5[:], t2[:], t3[:])
                    nc.scalar.activation(f6[:], t3[:], Square)
                    nc.vector.tensor_mul(f7[:], t3[:], t4[:])
                    nc.scalar.activation(f8[:], t4[:], Square)
                    chunks = [t1r, f2r, f3, f4r, f5, f6, f7, f8]
                    if ib < N_IB - 1:
                        order = [(j, oc) for j in range(N_J) for oc in range(4)]
                    else:
                        # oc-major on the last row-block: accumulation groups
                        # finish staggered -> evacuation overlaps matmuls
                        order = [(j, oc) for oc in range(4) for j in range(N_J)]
                    for j, oc in order:
                        nc.tensor.matmul(
                            ps[oc][:],
                            lhsT=w_chunk(ib, j, oc),
                            rhs=chunks[j][:],
                            start=(ib == 0 and j == 0),
                            stop=(ib == N_IB - 1 and j == N_J - 1),
                        )
                        if ib == N_IB - 1 and j == N_J - 1:
                            osb = opool.tile(
                                [128, B_TILE], F32, tag=f"osb{oc}", name=f"osb{oc}"
                            )
                            nc.scalar.activation(
                                osb[:], ps[oc][:], Identity,
                                bias=bias_t[:, oc : oc + 1],
                            )
                            out_eng = (
                                nc.sync if ph == N_PHASES - 1 else nc.gpsimd
                            )
                            out_eng.dma_start(
                                out=yt.ap()[oc * 128 : (oc + 1) * 128, bsl],
                                in_=osb[:],
                            )
    nc.compile()
    return nc


def _host_transform(cheby_coeffs):
    # Map Chebyshev coefficients onto the device phi basis:
    # phi = [T1, T1^2, T1*T2, T2^2, T2*T3, T3^2, T3*T4, T4^2] and a constant.
    # T_{2k} = 2*T_k^2 - 1, T_{m+n} = 2*T_m*T_n - T_{m-n} =>
    #   y = bias + (C1-C3-C5-C7)*T1 + sum_{d=2..8} 2*C_d * phi_{d-1}
    #   bias_o = sum_i (C0 - C2 - C4 - C6 - C8)
    C64 = cheby_coeffs.astype(np.float64)
    bias = (C64[..., 0] - C64[..., 2] - C64[..., 4] - C64[..., 6] - C64[..., 8]).sum(
        axis=0
    )
    W = np.empty((C_IN, C_OUT, N_J), np.float64)
    W[..., 0] = C64[..., 1] - C64[..., 3] - C64[..., 5] - C64[..., 7]
    for d in range(2, DEGREE + 1):
        W[..., d - 1] = 2.0 * C64[..., d]
    # [i, j*512+o]: per-partition-contiguous coefficient rows
    Wd = np.ascontiguousarray(
        W.transpose(0, 2, 1).reshape(C_IN, N_J * C_OUT).astype(np.float32)
    )
    bias_dev = np.ascontiguousarray(bias.reshape(4, 128).T.astype(np.float32))
    return Wd, bias_dev


def kernel(x, cheby_coeffs):
    x = np.asarray(x, dtype=np.float32)
    cheby_coeffs = np.asarray(cheby_coeffs, dtype=np.float32)
    if "nc" not in _CACHE:
        _CACHE["nc"] = _build()
    nc = _CACHE["nc"]

    Wd, bias_dev = _host_transform(cheby_coeffs)
    xT = np.ascontiguousarray(x.T)                       # [c_in, b]
    in_maps = [
        {
            "xt": np.ascontiguousarray(xT[:, c * NB : (c + 1) * NB]),
            "wmat": Wd,
            "biasv": bias_dev,
        }
        for c in range(N_CORES)
    ]
    res = run_bass_kernel_spmd(nc, in_maps, core_ids=list(range(N_CORES)))
    y = np.concatenate([res.results[c]["yt"].T for c in range(N_CORES)], axis=0)
    return y
